# revision 1
# baseline (speedup 1.0000x reference)
"""Binarized dilated conv + BatchNorm + tanh on 8 Trainium2 NeuronCores.

Math (matches the reference nn.Module):
    bx = sign(x); bw = sign(W)
    y  = conv(bx, bw, stride=1, padding=2, dilation=2)     # [N,256,56,56]
    out = tanh((y - mean_b) * rsqrt(var_b + eps) * gamma + beta)
with mean/var computed over the full batch (training-mode BN).

Distribution: data-parallel over the batch, 4 images per core; weights
replicated; BN (sum, sumsq) per channel all-reduced across the 8 cores.

Schedule (v2 — LDWEIGHTS-deduped, coc-pipelined BN):
  * sign(x) is written into a zero-padded 60x60 fp8 image; each dilated tap
    is a shifted DoubleRow matmul contracting both ci-halves at once.
  * k-outer loop per (image, coc): one weight load serves the 7 spatial
    tiles (7 PSUM banks); the per-matmul InstLdweights that tile-legalize
    emits are deduped post-hoc (consecutive identical loads removed), so
    the PE does 9 weight loads per image instead of 63.
  * PSUM->SBUF evacuation alternates ACT/DVE in bank order so banks recycle
    at matmul pace; bn_stats per tile on DVE.
  * BN is pipelined by output-channel half (coc): coc0's stats all-reduce
    and tanh run under coc1's conv; only coc1's reduce+tanh are exposed.
  * rsqrt(var+eps) via Newton on DVE (no ACT table switch; ACT stays on
    the Sign/Copy/Tanh table set the whole kernel).
  * x and W ship as bf16, output ships as bf16 (host casts back to f32):
    halves all DRAM traffic; sign/tanh are insensitive to the cast.
"""

import numpy as np
import ml_dtypes

import concourse.bass as bass
import concourse.mybir as mybir
import concourse.tile as tile
from concourse import bacc
from concourse import bass_utils

F32 = mybir.dt.float32
BF16 = mybir.dt.bfloat16
FP8 = mybir.dt.float8e4
AF = mybir.ActivationFunctionType

N_CORES = 8
N_TOTAL = 32  # full batch
NIMG = N_TOTAL // N_CORES  # images per core
C = 256
H = W = 56
HW = H * W
PAD = 2
PH = PW = H + 2 * PAD  # 60
P = 128
CHI = C // P  # 2 input-channel halves
COC = C // P  # 2 output-channel chunks
RCH = 8  # rows per spatial tile
RC = H // RCH  # 7 spatial tiles
NT = RCH * W  # 448 useful columns per tile
NTP = RCH * PW  # 480 streamed columns (8 padded rows)
NROW = PH + 1  # one spare row so the deepest shifted 480-read is in-bounds
HALF = H // 2  # sign() staging granularity: half images
EPS = 1e-5
# bx is binarized to {-0.5,+0.5} (BN cancels any consistent scale; a single
# DVE/GPSIMD tensor_scalar computes (x>=0)-0.5 in one pass, where a true
# +-1 sign would need two).  y is then scaled by 1/2, so match the
# reference's var+EPS with var' + EPS/4 and seed Newton around
# E[var(y)]/4 ~ 2160/4.
EPS_EFF = EPS / 4
RSQRT_SEED = 0.043  # ~1/sqrt(2160/4)
OUT_SHAPE = (N_TOTAL, C, H, W)


def _dedupe_ldweights(nc):
    """Remove consecutive InstLdweights with identical source APs.

    tile-legalize pairs every InstMatmult with its own InstLdweights even
    when the stationary operand is unchanged; on HW each DoubleRow load
    costs ~213 ns (256 columns), which made the baseline PE weight-load
    bound. Keeping only the first load of each identical run is safe: the
    paired matmuls carry the same data deps (their ins include the weights
    AP), and nothing writes w_bf after its initial binarize.
    """
    removed = 0
    for b in nc.m.functions[0].blocks:
        insts = b.instructions
        prev_key = None
        i = 0
        while i < len(insts):
            inst = insts[i]
            tn = type(inst).__name__
            if tn == "InstLdweights":
                key = str(inst.ins)
                if key == prev_key and inst.sync_info is None:
                    nxt = insts[i + 1] if i + 1 < len(insts) else None
                    if nxt is not None:
                        try:
                            nxt.merge_dependencies_from(inst)
                        except Exception:
                            pass
                    del insts[i]
                    removed += 1
                    continue
                prev_key = key
            elif tn != "InstMatmult" and getattr(inst, "engine", None) == mybir.EngineType.PE:
                prev_key = None  # other PE-queue inst invalidates the array
            i += 1
    return removed


def build(
    n_img=NIMG,
    collective=True,
    n_cores=N_CORES,
    fp8=True,  # kept for test.py compat; only the fp8 path exists
    n_rep=1,
    io_alias=False,
    phase="all",  # 'head' | 'conv' | 'all' — truncated builds for cost probing
    dedupe=True,
):
    """Emit + compile the per-core Bass program (see module docstring)."""
    nc = bacc.Bacc(
        "TRN2",
        target_bir_lowering=False,
        debug=False,
        num_devices=n_cores if collective else 1,
    )
    nio = 1 if io_alias else n_img
    x_d = nc.dram_tensor("x", [nio, C, HW], BF16, kind="ExternalInput").ap()
    wt_d = nc.dram_tensor("wt", [C, 9, C], BF16, kind="ExternalInput").ap()
    gamma_d = nc.dram_tensor("gamma", [C], F32, kind="ExternalInput").ap()
    beta_d = nc.dram_tensor("beta", [C], F32, kind="ExternalInput").ap()
    out_d = nc.dram_tensor("out", [nio, C, HW], BF16, kind="ExternalOutput").ap()

    with tile.TileContext(nc) as tc:
        with (
            tc.tile_pool(name="const", bufs=1) as const,
            tc.tile_pool(name="bx", bufs=1) as bxp,
            tc.tile_pool(name="ysb", bufs=1) as ysbp,
            tc.tile_pool(name="xs", bufs=2) as xsp,
            tc.tile_pool(name="psk", bufs=1, space="PSUM") as psk,
            tc.tile_pool(name="outp", bufs=3) as outp,
            tc.tile_pool(name="dram", bufs=1, space="DRAM") as dram,
        ):
            # ---- weights: load, binarize ----
            w_st = const.tile([P, CHI, 9, C], BF16)
            nc.sync.dma_start(
                out=w_st, in_=wt_d.rearrange("(chi p) k co -> p chi k co", p=P)
            )
            w_bf = const.tile([P, CHI, 9, C], FP8)
            nc.scalar.activation(out=w_bf, in_=w_st, func=AF.Sign)

            # ---- gamma/beta ----
            gamma_sb = const.tile([P, COC], F32)
            beta_sb = const.tile([P, COC], F32)
            nc.sync.dma_start(out=gamma_sb, in_=gamma_d.rearrange("(c p) -> p c", p=P))
            nc.sync.dma_start(out=beta_sb, in_=beta_d.rearrange("(c p) -> p c", p=P))

            def body():
                # ---- bx tiles + halo zeroing ----
                bx_tiles = [
                    bxp.tile([P, CHI, NROW, PW], FP8, tag=f"bx{i}", name=f"bx{i}")
                    for i in range(n_img)
                ]

                def zero_halo(i, eng):
                    # zero only the halo; the interior is overwritten by sign.
                    fl = bx_tiles[i].rearrange("p c h w -> p c (h w)")
                    eng.memset(fl[:, :, 0 : 2 * PW + 2], 0.0)
                    off = 2 * PW + 2 + H  # row 2, col 58
                    eng.memset(
                        fl[:, :, off : off + H * PW].rearrange(
                            "p c (h w) -> p c h w", w=PW
                        )[:, :, :, 0:4],
                        0.0,
                    )
                    eng.memset(fl[:, :, (H + 2) * PW + 2 : NROW * PW], 0.0)

                def stage_sign(i):
                    """DMA image i and binarize it to +-0.5 into its bx tile
                    in ONE DVE op (ACT keeps the PSUM evictions; GPSIMD
                    tensor_scalar measured ~108us/op — unusable).
                    Whole-image: a partially-signed image would gate some of
                    a set's banks mid-ladder and shuffle the LD runs."""
                    xr = x_d[0 if io_alias else i].rearrange(
                        "(chi p) hw -> p chi hw", p=P
                    )
                    xs = xsp.tile([P, CHI, HW], BF16, tag="xs")
                    nc.sync.dma_start(out=xs, in_=xr)
                    zero_halo(i, nc.vector)
                    # two row-chunks: rows <44 gate the A-set (rc0-3), the
                    # rest only the B-set — the A-gate lands ~1.5us earlier
                    for r0, r1 in ((0, 44), (44, H)):
                        nc.vector.tensor_scalar(
                            out=bx_tiles[i][
                                :, :, PAD + r0 : PAD + r1, PAD : PAD + W
                            ],
                            in0=xs.rearrange("p c (h w) -> p c h w", w=W)[
                                :, :, r0:r1, :
                            ],
                            scalar1=0.0,
                            scalar2=0.5,
                            op0=mybir.AluOpType.is_ge,
                            op1=mybir.AluOpType.subtract,
                        )

                # all signs up front on DVE, before any bn_stats queue in
                # front of them; sets consume images at ~6us apiece so sign
                # i is ~ready by the time image i's first ladder starts
                for i in range(n_img):
                    stage_sign(i)
                if phase == "head":
                    return

                # ---- per-core state for BN pipeline ----
                y_sb = ysbp.tile([P, n_img, COC, HW], BF16, tag="ysb")
                bnst = [
                    const.tile(
                        [P, n_img * RC, 6], F32, tag=f"bnst{c}", name=f"bnst{c}"
                    )
                    for c in range(COC)
                ]
                ab = {}  # coc -> (a_t, b_t)

                set_ord = [0]  # ping-pong between PSUM bank columns {0-3},{4-7}
                HNT = NT // 2

                def conv_set(tasks, coc, evict=None):
                    """One 9-tap weight ladder over a set of <=4 PSUM banks.

                    Each bank accumulates one (image, rc) tile; one weight
                    load per tap serves the whole set (runs stay adjacent for
                    the LDWEIGHTS dedup because the set's banks were fully
                    evacuated during the previous set's ladder — the
                    scheduler never has to run ahead). Eviction: two
                    half-copies per bank on ACT+DVE, then bn_stats on DVE.
                    """
                    col = 4 * (set_ord[0] % 2)
                    set_ord[0] += 1
                    pts = [
                        psk.tile(
                            [P, NT], F32, tag=f"pt{col + j}", name=f"pt{col + j}"
                        )
                        for j in range(len(tasks))
                    ]
                    for kh in range(3):
                        for kw in range(3):
                            k = kh * 3 + kw
                            lhsT = w_bf[:, :, k, coc * P : (coc + 1) * P]
                            # boustrophedon bank order: tap k+1's first
                            # matmul accumulates on tap k's last bank, so
                            # taps are dependency-chained and the scheduler
                            # cannot run ahead and split the LD runs
                            order = range(len(tasks))
                            if k % 2 == 1:
                                order = reversed(list(order))
                            for j in order:
                                i, rc = tasks[j]
                                rhs = bx_tiles[i][
                                    :,
                                    :,
                                    rc * RCH + 2 * kh : rc * RCH + 2 * kh + RCH,
                                    2 * kw : 2 * kw + W,
                                ]
                                nc.tensor.matmul(
                                    pts[j],
                                    lhsT,
                                    rhs,
                                    start=(k == 0),
                                    stop=(k == 8),
                                    perf_mode=mybir.MatmulPerfMode.DoubleRow,
                                )
                    for j, (i, rc) in enumerate(tasks):
                        h0w = rc * RCH * W
                        dst = y_sb[:, i, coc, h0w : h0w + NT]
                        if evict is nc.vector:
                            nc.vector.tensor_copy(out=dst, in_=pts[j])
                        else:
                            nc.scalar.activation(
                                out=dst, in_=pts[j], func=AF.Copy
                            )
                    for j, (i, rc) in enumerate(tasks):
                        nc.vector.bn_stats(
                            out=bnst[coc][:, i * RC + rc, :],
                            in_=y_sb[:, i, coc, rc * RCH * W : rc * RCH * W + NT],
                        )

                # two sets per (image, coc): {rc0-3} on bank column A and
                # {rc4-6} on column B — each ladder runs while the other
                # column's banks evacuate on ACT
                SETS = []
                for i in range(n_img):
                    SETS.append([(i, rc) for rc in range(4)])
                    SETS.append([(i, rc) for rc in range(4, RC)])

                def reduce_stats(coc):
                    """Aggregate per-tile stats -> per-core (mean, E[y^2]),
                    all-reduce across cores, then a/b via Newton rsqrt on
                    DVE (no ACT involvement)."""
                    stats = const.tile([P, 2], F32, tag=f"stats{coc}")
                    nc.vector.bn_aggr(out=stats, in_=bnst[coc])
                    msq = const.tile([P, 1], F32, tag=f"msq{coc}")
                    nc.vector.tensor_mul(
                        out=msq, in0=stats[:, 0:1], in1=stats[:, 0:1]
                    )
                    nc.vector.tensor_add(
                        out=stats[:, 1:2], in0=stats[:, 1:2], in1=msq
                    )
                    if collective:
                        b_in = dram.tile([P, 2], F32, tag=f"b_in{coc}")
                        b_out = dram.tile([P, 2], F32, tag=f"b_out{coc}")
                        nc.gpsimd.dma_start(out=b_in, in_=stats)
                        nc.gpsimd.collective_compute(
                            "AllReduce",
                            mybir.AluOpType.add,
                            replica_groups=[list(range(n_cores))],
                            ins=[b_in.opt()],
                            outs=[b_out.opt()],
                        )
                        stats_g = const.tile([P, 2], F32, tag=f"stats_g{coc}")
                        nc.gpsimd.dma_start(out=stats_g, in_=b_out)
                    else:
                        stats_g = stats

                    inv_n = (1.0 / n_cores) if collective else 1.0
                    mean_t = const.tile([P, 1], F32, tag=f"mean{coc}")
                    v_t = const.tile([P, 1], F32, tag=f"v{coc}")
                    # mean = sum/n; var = E2/n - mean^2; v = var + eps
                    nc.vector.tensor_scalar_mul(
                        out=mean_t, in0=stats_g[:, 0:1], scalar1=inv_n
                    )
                    nc.vector.tensor_mul(out=v_t, in0=mean_t, in1=mean_t)
                    nc.vector.scalar_tensor_tensor(
                        out=v_t,
                        in0=stats_g[:, 1:2],
                        scalar=inv_n,
                        in1=v_t,
                        op0=mybir.AluOpType.mult,
                        op1=mybir.AluOpType.subtract,
                    )
                    nc.vector.tensor_scalar_add(out=v_t, in0=v_t, scalar1=EPS_EFF)
                    # Newton rsqrt: r <- r*(1.5 - 0.5*v*r^2), 3 iters
                    r_t = const.tile([P, 1], F32, tag=f"r{coc}")
                    t_t = const.tile([P, 1], F32, tag=f"t{coc}")
                    nc.vector.memset(r_t, RSQRT_SEED)
                    for _ in range(3):
                        nc.vector.tensor_mul(out=t_t, in0=r_t, in1=r_t)
                        nc.vector.tensor_mul(out=t_t, in0=v_t, in1=t_t)
                        nc.vector.tensor_scalar(
                            out=t_t,
                            in0=t_t,
                            scalar1=-0.5,
                            scalar2=1.5,
                            op0=mybir.AluOpType.mult,
                            op1=mybir.AluOpType.add,
                        )
                        nc.vector.tensor_mul(out=r_t, in0=r_t, in1=t_t)
                    a_t = const.tile([P, 1], F32, tag=f"a{coc}")
                    b_t = const.tile([P, 1], F32, tag=f"b{coc}")
                    nc.vector.tensor_mul(
                        out=a_t, in0=gamma_sb[:, coc : coc + 1], in1=r_t
                    )
                    nc.vector.tensor_mul(out=b_t, in0=mean_t, in1=a_t)
                    nc.vector.tensor_sub(
                        out=b_t, in0=beta_sb[:, coc : coc + 1], in1=b_t
                    )
                    ab[coc] = (a_t, b_t)

                def tanh_store(i, coc):
                    """tanh(a*y+b) for one (image, coc) in a single ACT op,
                    then one contiguous bf16 DMA to DRAM."""
                    a_t, b_t = ab[coc]
                    ot = outp.tile([P, HW], BF16, tag="ot")
                    nc.scalar.activation(
                        out=ot,
                        in_=y_sb[:, i, coc, :],
                        func=AF.Tanh,
                        bias=b_t,
                        scale=a_t,
                    )
                    orr = out_d[0 if io_alias else i].rearrange(
                        "(c p) hw -> p c hw", p=P
                    )
                    nc.sync.dma_start(out=orr[:, coc, :], in_=ot)

                # ---- coc0 conv ----
                for s, tasks in enumerate(SETS):
                    conv_set(tasks, 0)
                if phase == "conv":
                    return
                reduce_stats(0)
                # ---- coc1 conv; last coc0 tanh chunks in late windows ----
                tanh_at = {6: 0, 7: 1}  # set idx -> image to tanh
                for s, tasks in enumerate(SETS):
                    conv_set(tasks, 1, evict=nc.vector if s % 2 else None)
                    if s in tanh_at:
                        tanh_store(tanh_at[s], 0)
                reduce_stats(1)
                for i in range(2, n_img):
                    tanh_store(i, 0)
                for i in range(n_img):
                    tanh_store(i, 1)

            for _ in range(n_rep):
                body()

    if dedupe:
        _dedupe_ldweights(nc)
    nc.compile()
    return nc


_CACHE: dict = {}


def _built():
    if "nc" not in _CACHE:
        _CACHE["nc"] = build()
    return _CACHE["nc"]


def make_in_maps(x, W, gamma, beta):
    x = (
        np.ascontiguousarray(np.asarray(x, dtype=np.float32))
        .astype(ml_dtypes.bfloat16)
        .reshape(N_CORES, NIMG, C, HW)
    )
    wt = (
        np.ascontiguousarray(np.asarray(W, dtype=np.float32).transpose(1, 2, 3, 0))
        .astype(ml_dtypes.bfloat16)
        .reshape(C, 9, C)
    )
    gamma = np.ascontiguousarray(np.asarray(gamma, dtype=np.float32))
    beta = np.ascontiguousarray(np.asarray(beta, dtype=np.float32))
    return [
        {"x": x[c], "wt": wt, "gamma": gamma, "beta": beta} for c in range(N_CORES)
    ]


def kernel(x, W, gamma, beta):
    nc = _built()
    in_maps = make_in_maps(x, W, gamma, beta)
    res = bass_utils.run_bass_kernel_spmd(nc, in_maps, core_ids=list(range(N_CORES)))
    out = np.stack([res.results[c]["out"] for c in range(N_CORES)])
    return out.astype(np.float32).reshape(OUT_SHAPE)

